# revision 12
# baseline (speedup 1.0000x reference)
"""Attention2d Trainium2 kernel.

Reference computation (per sample b):
  K = Wk @ x + bk;  Q = Wq @ x + bq;  V = Wv @ x + bv     (x: [128, 1024])
  per head h (32 channels):  att[k,q] = scale * K_h[:,k].Q_h[:,q] + rel_h[k,q]
  P = softmax_k(att);  out_h = V_h @ P;  y = Wu @ out + bu

Kernel strategy (8 NeuronCores, data-parallel over batch, 2 samples/core):
  - host: transpose weights (lhsT layouts), fold `scale` into Wq, gather
    rel = pos_enc[:, idx] -> bf16, fold bv/bu into one final bias (softmax
    column-sums are 1, so V-bias passes through attention unchanged), drop
    bk (constant-in-k shift, softmax-invariant).
  - att computed in [k_part, q_free] layout; rel added by an identity
    matmul accumulating into the same PSUM bank; exp on ScalarE.
  - softmax denominator D[q] via an appended ones-column in the V^T
    stationary operand (row 32 of the 2nd matmul output), division applied
    after the 2nd matmul via a selector-matmul partition-broadcast of 1/D.
  - all matmuls run as float32r (full-rate fp32 streaming on PE).
"""

import os
import sys
import types

sys.path.insert(0, "/opt/trn_rl_repo")

import numpy as np
import ml_dtypes

import concourse.bass as bass
import concourse.tile as tile
from concourse import bacc, mybir
from concourse import bass_utils
from concourse.bass import ds, ts

F32 = mybir.dt.float32
F32R = mybir.dt.float32r
BF16 = mybir.dt.bfloat16
AF = mybir.ActivationFunctionType

B, E, H, NY, NX = 16, 128, 4, 32, 32
N = NY * NX          # 1024
HC = E // H          # 32
NCORES = 8
BPC = B // NCORES    # 2 samples per core
NT = N // 128        # 8 k-tiles
SCALE = HC ** -0.5

LAST_RESULT = None   # BassKernelResults of the most recent run (for test.py)

_CACHE = {}


def _ensure_ntff_hook():
    """Register the axon NTFF profile hook that trn_boot couldn't install
    (the image lacks antenv.axon_hooks). Only needed when tracing."""
    if "antenv.axon_hooks" in sys.modules:
        return
    mod = types.ModuleType("antenv.axon_hooks")
    holder = [None]
    mod.set_axon_ntff_profile_hook = lambda h: holder.__setitem__(0, h)
    mod.get_axon_ntff_profile_hook = lambda: holder[0]
    sys.modules["antenv.axon_hooks"] = mod
    try:
        from trn_agent_boot.trn_boot import _ntff_profile_via_ctypes
        mod.set_axon_ntff_profile_hook(
            _ntff_profile_via_ctypes("/opt/axon/libaxon_pjrt.so")
        )
    except Exception:
        pass


def _rel_indices(ny, nx):
    y = np.arange(ny)
    x = np.arange(nx)
    y1, x1, y2, x2 = np.meshgrid(y, x, y, x, indexing="ij")
    idx = (y1 - y2 + ny - 1) * (2 * nx - 1) + (x1 - x2 + nx - 1)
    return idx.reshape(ny * nx, ny * nx)


def _build():
    """Build + bacc-compile the per-core program (cached)."""
    stage = int(os.environ.get("KSTAGE", "4"))
    key = ("nc", stage, os.environ.get("KSUB", "4"))
    if key in _CACHE:
        return _CACHE[key]

    nc = bacc.Bacc("TRN2", target_bir_lowering=False, debug=False,
                   num_devices=NCORES)

    d_x2 = nc.dram_tensor("x2", [BPC, E, N], F32R, kind="ExternalInput")
    d_wkT = nc.dram_tensor("wkT", [E, E], F32R, kind="ExternalInput")
    d_wqT = nc.dram_tensor("wqT", [E, E], F32R, kind="ExternalInput")
    d_wvT = nc.dram_tensor("wvT", [E, E], F32R, kind="ExternalInput")
    d_wuT = nc.dram_tensor("wuT", [E, E], F32R, kind="ExternalInput")
    d_bq = nc.dram_tensor("bqv", [E, 1], F32, kind="ExternalInput")
    d_bf = nc.dram_tensor("bfv", [E, 1], F32, kind="ExternalInput")
    d_rel = nc.dram_tensor("relb", [H, NT, 128, N], BF16, kind="ExternalInput")
    d_id = nc.dram_tensor("ident", [128, 128], BF16, kind="ExternalInput")
    d_sel = nc.dram_tensor("sel4", [128, E], F32R, kind="ExternalInput")
    d_y2 = nc.dram_tensor("y2", [BPC, E, N], F32, kind="ExternalOutput")

    def r(ap):
        return ap.bitcast(F32R)

    with nc.allow_low_precision(reason="fp32r matmul operand tiles"), \
         tile.TileContext(nc) as tc:
        with (
            tc.tile_pool(name="const", bufs=1) as const,
            tc.tile_pool(name="persist", bufs=1) as persist,
            tc.tile_pool(name="xp", bufs=1) as xp,
            tc.tile_pool(name="relp", bufs=2) as relp,
            tc.tile_pool(name="Ep", bufs=3) as Ep,
            tc.tile_pool(name="yp", bufs=1) as yp,
            tc.tile_pool(name="ps", bufs=4, space="PSUM") as ps,
        ):
            wk_sb = const.tile([E, E], F32R, tag="wk")
            wq_sb = const.tile([E, E], F32R, tag="wq")
            wv_sb = const.tile([E, E], F32R, tag="wv")
            wu_sb = const.tile([E, E], F32R, tag="wu")
            id_sb = const.tile([128, 128], BF16, tag="id")
            sel_sb = const.tile([128, E], F32R, tag="sel")
            bq_sb = const.tile([E, 1], F32, tag="bq")
            bf_sb = const.tile([E, 1], F32, tag="bf")
            nc.sync.dma_start(wk_sb[:], d_wkT.ap()[:])
            nc.sync.dma_start(wq_sb[:], d_wqT.ap()[:])
            nc.sync.dma_start(wv_sb[:], d_wvT.ap()[:])
            nc.sync.dma_start(wu_sb[:], d_wuT.ap()[:])
            nc.sync.dma_start(id_sb[:], d_id.ap()[:])
            nc.sync.dma_start(sel_sb[:], d_sel.ap()[:])
            nc.sync.dma_start(bq_sb[:], d_bq.ap()[:])
            nc.sync.dma_start(bf_sb[:], d_bf.ap()[:])

            K_sb, Q_sb, VT_sb, out_sb, R_sb = {}, {}, {}, {}, {}
            for b in range(BPC):
                K_sb[b] = persist.tile([E, N], F32R, tag=f"K{b}", name=f"K{b}")
                Q_sb[b] = persist.tile([E, N], F32R, tag=f"Q{b}", name=f"Q{b}")
                VT_sb[b] = persist.tile([128, NT, H, HC + 1], F32R, tag=f"VT{b}", name=f"VT{b}")
                out_sb[b] = persist.tile([E, N], F32R, tag=f"O{b}", name=f"O{b}")
                R_sb[b] = persist.tile([128, N], F32R, tag=f"R{b}", name=f"R{b}")

            # ---- projections ----
            for b in range(BPC):
                x_sb = xp.tile([E, N], F32R, tag="x")
                nc.sync.dma_start(x_sb[:], d_x2.ap()[b])
                nc.vector.memset(VT_sb[b][:].bitcast(F32), 1.0)
                nc.vector.memset(R_sb[b][:].bitcast(F32), 0.0)
                for j in range(2):
                    js = ds(512 * j, 512)
                    pk = ps.tile([128, 512], F32, tag="ps")
                    nc.tensor.matmul(pk[:], wk_sb[:], x_sb[:, js],
                                     start=True, stop=True)
                    nc.vector.tensor_copy(K_sb[b][:, js], pk[:])
                    pq = ps.tile([128, 512], F32, tag="ps")
                    nc.tensor.matmul(pq[:], wq_sb[:], x_sb[:, js],
                                     start=True, stop=True)
                    nc.vector.tensor_scalar_add(Q_sb[b][:, js], pq[:], bq_sb[:])
                for t in range(NT):
                    pv = ps.tile([128, 128], F32, tag="ps")
                    nc.tensor.matmul(pv[:], x_sb[:, ts(t, 128)], wv_sb[:],
                                     start=True, stop=True)
                    nc.vector.tensor_copy(
                        VT_sb[b][:, t, :, 0:HC],
                        pv[:].rearrange("p (h c) -> p h c", h=H),
                    )

            # ---- attention, head pairs ----
            for p in range(2 if stage >= 2 else 0):
                hs = (2 * p, 2 * p + 1)
                rel_t = {}
                for h in hs:
                    rel_t[h] = relp.tile([128, NT, N], BF16, tag="rel", name=f"rel{h}")
                    for t in range(NT):
                        nc.sync.dma_start(rel_t[h][:, t, :], d_rel.ap()[h, t])
                for b in range(BPC):
                    Et = {h: Ep.tile([128, NT, N], F32R, tag="E", name=f"E{h}") for h in hs}
                    for t in range(NT):
                        for h in hs:
                            pa = ps.tile([128, N], F32, tag="ps")
                            for j in range(2):
                                js = ds(512 * j, 512)
                                nc.tensor.matmul(
                                    pa[:, js],
                                    K_sb[b][ds(HC * h, HC), ts(t, 128)],
                                    Q_sb[b][ds(HC * h, HC), js],
                                    start=True, stop=False,
                                    tile_position=(HC * h, 0),
                                )
                                nc.tensor.matmul(
                                    pa[:, js], id_sb[:], rel_t[h][:, t, js],
                                    start=False, stop=True,
                                )
                            nc.scalar.activation(Et[h][:, t, :], pa[:], AF.Exp)
                    # second matmul: out_h^num / D, heads separately
                    for h in (hs if stage >= 3 else ()):
                        po = ps.tile([HC + 1, N], F32, tag="ps")
                        for j in range(2):
                            js = ds(512 * j, 512)
                            for t in range(NT):
                                nc.tensor.matmul(
                                    po[:, js],
                                    VT_sb[b][:, t, h, :],
                                    Et[h][:, t, js],
                                    start=(t == 0), stop=(t == NT - 1),
                                )
                        nc.vector.reciprocal(R_sb[b][ds(32 * h, 1), :], po[HC:HC + 1, :])
                        nc.vector.tensor_copy(out_sb[b][ds(HC * h, HC), :],
                                              po[0:HC, :])

            # ---- divide + output projection ----
            sub = int(os.environ.get("KSUB", "4"))
            for b in range(BPC if stage >= 4 else 0):
                pbc = ps.tile([128, N], F32, tag="ps")
                if sub >= 1:
                    for j in range(2):
                        js = ds(512 * j, 512)
                        nc.tensor.matmul(pbc[:, js], sel_sb[:], R_sb[b][:, js],
                                         start=True, stop=True)
                if sub >= 2:
                    nc.vector.tensor_mul(out_sb[b][:], out_sb[b][:], pbc[:])
                py = ps.tile([128, N], F32, tag="ps")
                if sub >= 3:
                    for j in range(2):
                        js = ds(512 * j, 512)
                        nc.tensor.matmul(py[:, js], wu_sb[:], out_sb[b][:, js],
                                         start=True, stop=True)
                    y_sb = yp.tile([E, N], F32, tag="y")
                    nc.vector.tensor_scalar_add(y_sb[:], py[:], bf_sb[:])
                    nc.sync.dma_start(d_y2.ap()[b], y_sb[:])

            if stage < 4 or int(os.environ.get("KSUB", "4")) < 3:
                for b in range(BPC):
                    nc.sync.dma_start(d_y2.ap()[b], K_sb[b][:].bitcast(F32))

    nc.compile()
    _CACHE[key] = nc
    return nc


def kernel(x, Wk, bk, Wq, bq, Wv, bv, Wu, bu, pos_enc):
    global LAST_RESULT
    x = np.ascontiguousarray(np.asarray(x, np.float32))
    Wk = np.asarray(Wk, np.float32)
    Wq = np.asarray(Wq, np.float32)
    Wv = np.asarray(Wv, np.float32)
    Wu = np.asarray(Wu, np.float32)
    bq = np.asarray(bq, np.float32)
    bv = np.asarray(bv, np.float32)
    bu = np.asarray(bu, np.float32)
    pos_enc = np.asarray(pos_enc, np.float32)

    wkT = np.ascontiguousarray(Wk.T)
    wqT = np.ascontiguousarray((Wq * SCALE).T)
    wvT = np.ascontiguousarray(Wv.T)
    wuT = np.ascontiguousarray(Wu.T)
    bqv = np.ascontiguousarray((bq * SCALE).reshape(E, 1))
    bfv = np.ascontiguousarray((Wu @ bv + bu).reshape(E, 1))

    idx = _rel_indices(NY, NX)
    rel = pos_enc[:, idx]                         # (H, N, N) fp32
    relb = np.ascontiguousarray(
        rel.reshape(H, NT, 128, N).astype(ml_dtypes.bfloat16))
    ident = np.eye(128, dtype=ml_dtypes.bfloat16)
    sel4 = np.zeros((128, E), np.float32)
    for h in range(H):
        sel4[32 * h, HC * h:HC * (h + 1)] = 1.0

    nc = _build()

    common = dict(wkT=wkT, wqT=wqT, wvT=wvT, wuT=wuT, bqv=bqv, bfv=bfv,
                  relb=relb, ident=ident, sel4=sel4)
    in_maps = []
    xr = x.reshape(B, E, N)
    for c in range(NCORES):
        m = dict(common)
        m["x2"] = np.ascontiguousarray(xr[BPC * c:BPC * (c + 1)])
        in_maps.append(m)

    trace = os.environ.get("BASS_TRACE", "") not in ("", "0")
    if trace:
        _ensure_ntff_hook()
    res = bass_utils.run_bass_kernel_spmd(
        nc, in_maps, core_ids=list(range(NCORES)), trace=trace)
    LAST_RESULT = res

    y = np.empty((B, E, N), np.float32)
    for c in range(NCORES):
        y[BPC * c:BPC * (c + 1)] = res.results[c]["y2"]
    return y.reshape(B, E, NY, NX)


# revision 14
# speedup vs baseline: 1.6626x; 1.6626x over previous
"""Attention2d Trainium2 kernel.

Reference computation (per sample b):
  K = Wk @ x + bk;  Q = Wq @ x + bq;  V = Wv @ x + bv     (x: [128, 1024])
  per head h (32 channels):  att[k,q] = scale * K_h[:,k].Q_h[:,q] + rel_h[k,q]
  P = softmax_k(att);  out_h = V_h @ P;  y = Wu @ out + bu

Kernel strategy (8 NeuronCores, data-parallel over batch, 2 samples/core):
  - host: transpose weights (lhsT layouts), fold `scale` into Wq, gather
    rel = pos_enc[:, idx] -> bf16, fold bv/bu into one final bias (softmax
    column-sums are 1, so V-bias passes through attention unchanged), drop
    bk (constant-in-k shift, softmax-invariant).
  - att computed in [k_part, q_free] layout; rel added by an identity
    matmul accumulating into the same PSUM bank; exp on ScalarE.
  - softmax denominator D[q] via an appended ones-column in the V^T
    stationary operand (row 32 of the 2nd matmul output), division applied
    after the 2nd matmul via a selector-matmul partition-broadcast of 1/D.
  - all matmuls run as float32r (full-rate fp32 streaming on PE).
"""

import os
import sys
import types

sys.path.insert(0, "/opt/trn_rl_repo")

import numpy as np
import ml_dtypes

import concourse.bass as bass
import concourse.tile as tile
from concourse import bacc, mybir
from concourse import bass_utils
from concourse.bass import ds, ts

F32 = mybir.dt.float32
F32R = mybir.dt.float32r
F16 = mybir.dt.float16
BF16 = mybir.dt.bfloat16
AF = mybir.ActivationFunctionType

B, E, H, NY, NX = 16, 128, 4, 32, 32
N = NY * NX          # 1024
HC = E // H          # 32
NCORES = 8
BPC = B // NCORES    # 2 samples per core
NT = N // 128        # 8 k-tiles
SCALE = HC ** -0.5

LAST_RESULT = None   # BassKernelResults of the most recent run (for test.py)

_CACHE = {}


def _ensure_ntff_hook():
    """Register the axon NTFF profile hook that trn_boot couldn't install
    (the image lacks antenv.axon_hooks). Only needed when tracing."""
    if "antenv.axon_hooks" in sys.modules:
        return
    mod = types.ModuleType("antenv.axon_hooks")
    holder = [None]
    mod.set_axon_ntff_profile_hook = lambda h: holder.__setitem__(0, h)
    mod.get_axon_ntff_profile_hook = lambda: holder[0]
    sys.modules["antenv.axon_hooks"] = mod
    try:
        from trn_agent_boot.trn_boot import _ntff_profile_via_ctypes
        mod.set_axon_ntff_profile_hook(
            _ntff_profile_via_ctypes("/opt/axon/libaxon_pjrt.so")
        )
    except Exception:
        pass


def _rel_indices(ny, nx):
    y = np.arange(ny)
    x = np.arange(nx)
    y1, x1, y2, x2 = np.meshgrid(y, x, y, x, indexing="ij")
    idx = (y1 - y2 + ny - 1) * (2 * nx - 1) + (x1 - x2 + nx - 1)
    return idx.reshape(ny * nx, ny * nx)


def _build():
    """Build + bacc-compile the per-core program (cached)."""
    stage = int(os.environ.get("KSTAGE", "4"))
    key = ("nc", stage, os.environ.get("KSUB", "4"))
    if key in _CACHE:
        return _CACHE[key]

    nc = bacc.Bacc("TRN2", target_bir_lowering=False, debug=False,
                   num_devices=NCORES)

    d_x2 = nc.dram_tensor("x2", [BPC, E, N], F16, kind="ExternalInput")
    d_wkT = nc.dram_tensor("wkT", [E, E], F16, kind="ExternalInput")
    d_wqT = nc.dram_tensor("wqT", [E, E], F16, kind="ExternalInput")
    d_wvT = nc.dram_tensor("wvT", [E, E], F16, kind="ExternalInput")
    d_wuT = nc.dram_tensor("wuT", [E, E], F16, kind="ExternalInput")
    d_bq = nc.dram_tensor("bqv", [E, 1], F32, kind="ExternalInput")
    d_bf = nc.dram_tensor("bfv", [E, 1], F32, kind="ExternalInput")
    d_rel = nc.dram_tensor("relb", [H, NT, 128, N], F16, kind="ExternalInput")
    d_id = nc.dram_tensor("ident", [128, 128], F16, kind="ExternalInput")
    d_sel = nc.dram_tensor("sel4", [128, E], F16, kind="ExternalInput")
    d_y2 = nc.dram_tensor("y2", [BPC, E, N], F32, kind="ExternalOutput")

    def r(ap):
        return ap.bitcast(F32R)

    with nc.allow_low_precision(reason="fp32r matmul operand tiles"), \
         tile.TileContext(nc) as tc:
        with (
            tc.tile_pool(name="const", bufs=1) as const,
            tc.tile_pool(name="persist", bufs=1) as persist,
            tc.tile_pool(name="xp", bufs=1) as xp,
            tc.tile_pool(name="relp", bufs=2) as relp,
            tc.tile_pool(name="Ep", bufs=3) as Ep,
            tc.tile_pool(name="yp", bufs=1) as yp,
            tc.tile_pool(name="ps", bufs=4, space="PSUM") as ps,
        ):
            wk_sb = const.tile([E, E], F16, tag="wk")
            wq_sb = const.tile([E, E], F16, tag="wq")
            wv_sb = const.tile([E, E], F16, tag="wv")
            wu_sb = const.tile([E, E], F16, tag="wu")
            id_sb = const.tile([128, 128], F16, tag="id")
            sel_sb = const.tile([128, E], F16, tag="sel")
            bq_sb = const.tile([E, 1], F32, tag="bq")
            bf_sb = const.tile([E, 1], F32, tag="bf")
            nc.sync.dma_start(wk_sb[:], d_wkT.ap()[:])
            nc.sync.dma_start(wq_sb[:], d_wqT.ap()[:])
            nc.sync.dma_start(wv_sb[:], d_wvT.ap()[:])
            nc.sync.dma_start(wu_sb[:], d_wuT.ap()[:])
            nc.sync.dma_start(id_sb[:], d_id.ap()[:])
            nc.sync.dma_start(sel_sb[:], d_sel.ap()[:])
            nc.sync.dma_start(bq_sb[:], d_bq.ap()[:])
            nc.sync.dma_start(bf_sb[:], d_bf.ap()[:])

            K_sb, Q_sb, VT_sb, out_sb, R_sb, R32_sb, RD_sb = {}, {}, {}, {}, {}, {}, {}
            for b in range(BPC):
                K_sb[b] = persist.tile([E, N], F16, tag=f"K{b}", name=f"K{b}")
                Q_sb[b] = persist.tile([E, N], F16, tag=f"Q{b}", name=f"Q{b}")
                VT_sb[b] = persist.tile([128, NT, H, HC + 1], F16, tag=f"VT{b}", name=f"VT{b}")
                out_sb[b] = persist.tile([E, N], F16, tag=f"O{b}", name=f"O{b}")
                R_sb[b] = persist.tile([128, N], F16, tag=f"R{b}", name=f"R{b}")
                R32_sb[b] = persist.tile([128, N], F32, tag=f"R32{b}", name=f"R32{b}")
                RD_sb[b] = persist.tile([128, N], F32, tag=f"RD{b}", name=f"RD{b}")

            # ---- projections ----
            for b in range(BPC):
                x_sb = xp.tile([E, N], F16, tag="x")
                nc.sync.dma_start(x_sb[:], d_x2.ap()[b])
                nc.vector.memset(VT_sb[b][:], 1.0)
                nc.vector.memset(R_sb[b][:], 0.0)
                nc.vector.memset(RD_sb[b][:], 1.0)
                for j in range(2):
                    js = ds(512 * j, 512)
                    pk = ps.tile([128, 512], F32, tag="ps")
                    nc.tensor.matmul(pk[:], wk_sb[:], x_sb[:, js],
                                     start=True, stop=True)
                    nc.vector.tensor_copy(K_sb[b][:, js], pk[:])
                    pq = ps.tile([128, 512], F32, tag="ps")
                    nc.tensor.matmul(pq[:], wq_sb[:], x_sb[:, js],
                                     start=True, stop=True)
                    nc.vector.tensor_scalar_add(Q_sb[b][:, js], pq[:], bq_sb[:])
                for t in range(NT):
                    pv = ps.tile([128, 128], F32, tag="ps")
                    nc.tensor.matmul(pv[:], x_sb[:, ts(t, 128)], wv_sb[:],
                                     start=True, stop=True)
                    nc.vector.tensor_copy(
                        VT_sb[b][:, t, :, 0:HC],
                        pv[:].rearrange("p (h c) -> p h c", h=H),
                    )

            # ---- attention, head pairs ----
            for p in range(2 if stage >= 2 else 0):
                hs = (2 * p, 2 * p + 1)
                rel_t = {}
                for h in hs:
                    rel_t[h] = relp.tile([128, NT, N], F16, tag="rel", name=f"rel{h}")
                    for t in range(NT):
                        nc.sync.dma_start(rel_t[h][:, t, :], d_rel.ap()[h, t])
                for b in range(BPC):
                    Et = {h: Ep.tile([128, NT, N], F16, tag="E", name=f"E{h}") for h in hs}
                    for t in range(NT):
                        for h in hs:
                            pa = ps.tile([128, N], F32, tag="ps")
                            for j in range(2):
                                js = ds(512 * j, 512)
                                nc.tensor.matmul(
                                    pa[:, js],
                                    K_sb[b][ds(HC * h, HC), ts(t, 128)],
                                    Q_sb[b][ds(HC * h, HC), js],
                                    start=True, stop=False,
                                    tile_position=(HC * h, 0),
                                )
                                nc.tensor.matmul(
                                    pa[:, js], id_sb[:], rel_t[h][:, t, js],
                                    start=False, stop=True,
                                )
                            nc.scalar.activation(Et[h][:, t, :], pa[:], AF.Exp)
                    # second matmul: out_h^num / D, heads separately
                    for h in (hs if stage >= 3 else ()):
                        po = ps.tile([HC + 1, N], F32, tag="ps")
                        for j in range(2):
                            js = ds(512 * j, 512)
                            for t in range(NT):
                                nc.tensor.matmul(
                                    po[:, js],
                                    VT_sb[b][:, t, h, :],
                                    Et[h][:, t, js],
                                    start=(t == 0), stop=(t == NT - 1),
                                )
                        nc.vector.tensor_copy(RD_sb[b][ds(32 * h, 1), :], po[HC:HC + 1, :])
                        nc.vector.tensor_copy(out_sb[b][ds(HC * h, HC), :],
                                              po[0:HC, :])

            # ---- divide + output projection ----
            sub = int(os.environ.get("KSUB", "4"))
            for b in range(BPC if stage >= 4 else 0):
                nc.vector.reciprocal_approx_fast(out=R32_sb[b][:], in_=RD_sb[b][:])
                nc.vector.tensor_copy(R_sb[b][:], R32_sb[b][:])
                pbc = ps.tile([128, N], F32, tag="ps")
                if sub >= 1:
                    for j in range(2):
                        js = ds(512 * j, 512)
                        nc.tensor.matmul(pbc[:, js], sel_sb[:], R_sb[b][:, js],
                                         start=True, stop=True)
                if sub >= 2:
                    nc.vector.tensor_mul(out_sb[b][:], out_sb[b][:], pbc[:])
                py = ps.tile([128, N], F32, tag="ps")
                if sub >= 3:
                    for j in range(2):
                        js = ds(512 * j, 512)
                        nc.tensor.matmul(py[:, js], wu_sb[:], out_sb[b][:, js],
                                         start=True, stop=True)
                    y_sb = yp.tile([E, N], F32, tag="y")
                    nc.vector.tensor_scalar_add(y_sb[:], py[:], bf_sb[:])
                    nc.sync.dma_start(d_y2.ap()[b], y_sb[:])

            if stage < 4 or int(os.environ.get("KSUB", "4")) < 3:
                for b in range(BPC):
                    nc.gpsimd.dma_start(d_y2.ap()[b], K_sb[b][:])

    nc.compile()
    _CACHE[key] = nc
    return nc


def kernel(x, Wk, bk, Wq, bq, Wv, bv, Wu, bu, pos_enc):
    global LAST_RESULT
    x = np.ascontiguousarray(np.asarray(x, np.float32))
    Wk = np.asarray(Wk, np.float32)
    Wq = np.asarray(Wq, np.float32)
    Wv = np.asarray(Wv, np.float32)
    Wu = np.asarray(Wu, np.float32)
    bq = np.asarray(bq, np.float32)
    bv = np.asarray(bv, np.float32)
    bu = np.asarray(bu, np.float32)
    pos_enc = np.asarray(pos_enc, np.float32)

    wkT = np.ascontiguousarray(Wk.T.astype(np.float16))
    wqT = np.ascontiguousarray((Wq * SCALE).T.astype(np.float16))
    wvT = np.ascontiguousarray(Wv.T.astype(np.float16))
    wuT = np.ascontiguousarray(Wu.T.astype(np.float16))
    bqv = np.ascontiguousarray((bq * SCALE).reshape(E, 1))
    bfv = np.ascontiguousarray((Wu @ bv + bu).reshape(E, 1))

    idx = _rel_indices(NY, NX)
    rel = pos_enc[:, idx]                         # (H, N, N) fp32
    relb = np.ascontiguousarray(
        rel.reshape(H, NT, 128, N).astype(np.float16))
    ident = np.eye(128, dtype=np.float16)
    sel4 = np.zeros((128, E), np.float16)
    for h in range(H):
        sel4[32 * h, HC * h:HC * (h + 1)] = 1.0

    nc = _build()

    common = dict(wkT=wkT, wqT=wqT, wvT=wvT, wuT=wuT, bqv=bqv, bfv=bfv,
                  relb=relb, ident=ident, sel4=sel4)
    in_maps = []
    xr = x.reshape(B, E, N)
    for c in range(NCORES):
        m = dict(common)
        m["x2"] = np.ascontiguousarray(xr[BPC * c:BPC * (c + 1)].astype(np.float16))
        in_maps.append(m)

    trace = os.environ.get("BASS_TRACE", "") not in ("", "0")
    if trace:
        _ensure_ntff_hook()
    res = bass_utils.run_bass_kernel_spmd(
        nc, in_maps, core_ids=list(range(NCORES)), trace=trace)
    LAST_RESULT = res

    y = np.empty((B, E, N), np.float32)
    for c in range(NCORES):
        y[BPC * c:BPC * (c + 1)] = res.results[c]["y2"]
    return y.reshape(B, E, NY, NX)


# revision 19
# speedup vs baseline: 1.6969x; 1.0207x over previous
"""Attention2d Trainium2 kernel.

Reference computation (per sample b):
  K = Wk @ x + bk;  Q = Wq @ x + bq;  V = Wv @ x + bv     (x: [128, 1024])
  per head h (32 channels):  att[k,q] = scale * K_h[:,k].Q_h[:,q] + rel_h[k,q]
  P = softmax_k(att);  out_h = V_h @ P;  y = Wu @ out + bu

Kernel strategy (8 NeuronCores, data-parallel over batch, 2 samples/core):
  - host: transpose weights (lhsT layouts), fold `scale` into Wq, gather
    rel = pos_enc[:, idx] -> bf16, fold bv/bu into one final bias (softmax
    column-sums are 1, so V-bias passes through attention unchanged), drop
    bk (constant-in-k shift, softmax-invariant).
  - att computed in [k_part, q_free] layout; rel added by an identity
    matmul accumulating into the same PSUM bank; exp on ScalarE.
  - softmax denominator D[q] via an appended ones-column in the V^T
    stationary operand (row 32 of the 2nd matmul output), division applied
    after the 2nd matmul via a selector-matmul partition-broadcast of 1/D.
  - all matmuls run as float32r (full-rate fp32 streaming on PE).
"""

import os
import sys
import types

sys.path.insert(0, "/opt/trn_rl_repo")

import numpy as np
import ml_dtypes

import concourse.bass as bass
import concourse.tile as tile
from concourse import bacc, mybir
from concourse import bass_utils
from concourse.bass import ds, ts

F32 = mybir.dt.float32
F32R = mybir.dt.float32r
F16 = mybir.dt.float16
BF16 = mybir.dt.bfloat16
AF = mybir.ActivationFunctionType

B, E, H, NY, NX = 16, 128, 4, 32, 32
N = NY * NX          # 1024
HC = E // H          # 32
NCORES = 8
BPC = B // NCORES    # 2 samples per core
NT = N // 128        # 8 k-tiles
SCALE = HC ** -0.5

LAST_RESULT = None   # BassKernelResults of the most recent run (for test.py)

_CACHE = {}


def _ensure_ntff_hook():
    """Register the axon NTFF profile hook that trn_boot couldn't install
    (the image lacks antenv.axon_hooks). Only needed when tracing."""
    if "antenv.axon_hooks" in sys.modules:
        return
    mod = types.ModuleType("antenv.axon_hooks")
    holder = [None]
    mod.set_axon_ntff_profile_hook = lambda h: holder.__setitem__(0, h)
    mod.get_axon_ntff_profile_hook = lambda: holder[0]
    sys.modules["antenv.axon_hooks"] = mod
    try:
        from trn_agent_boot.trn_boot import _ntff_profile_via_ctypes
        mod.set_axon_ntff_profile_hook(
            _ntff_profile_via_ctypes("/opt/axon/libaxon_pjrt.so")
        )
    except Exception:
        pass


def _rel_indices(ny, nx):
    y = np.arange(ny)
    x = np.arange(nx)
    y1, x1, y2, x2 = np.meshgrid(y, x, y, x, indexing="ij")
    idx = (y1 - y2 + ny - 1) * (2 * nx - 1) + (x1 - x2 + nx - 1)
    return idx.reshape(ny * nx, ny * nx)


def _build():
    """Build + bacc-compile the per-core program (cached)."""
    stage = int(os.environ.get("KSTAGE", "4"))
    key = ("nc", stage, os.environ.get("KSUB", "4"))
    if key in _CACHE:
        return _CACHE[key]

    nc = bacc.Bacc("TRN2", target_bir_lowering=False, debug=False,
                   num_devices=NCORES)

    d_x2 = nc.dram_tensor("x2", [BPC, E, N], F16, kind="ExternalInput")
    d_wkT = nc.dram_tensor("wkT", [E, E], F16, kind="ExternalInput")
    d_wqT = nc.dram_tensor("wqT", [E, E], F16, kind="ExternalInput")
    d_wvT = nc.dram_tensor("wvT", [E, E], F16, kind="ExternalInput")
    d_wuT = nc.dram_tensor("wuT", [E, E], F16, kind="ExternalInput")
    d_bq = nc.dram_tensor("bqv", [E, 1], F32, kind="ExternalInput")
    d_bf = nc.dram_tensor("bfv", [E, 1], F32, kind="ExternalInput")
    d_rel = nc.dram_tensor("relb", [H, NT, 128, N], F16, kind="ExternalInput")
    d_id = nc.dram_tensor("ident", [128, 128], F16, kind="ExternalInput")
    d_sel = nc.dram_tensor("sel4", [128, E], F16, kind="ExternalInput")
    d_y2 = nc.dram_tensor("y2", [BPC, E, N], F32, kind="ExternalOutput")

    def r(ap):
        return ap.bitcast(F32R)

    with nc.allow_low_precision(reason="fp32r matmul operand tiles"), \
         tile.TileContext(nc) as tc:
        with (
            tc.tile_pool(name="const", bufs=1) as const,
            tc.tile_pool(name="persist", bufs=1) as persist,
            tc.tile_pool(name="xp", bufs=1) as xp,
            tc.tile_pool(name="relp", bufs=2) as relp,
            tc.tile_pool(name="Ep", bufs=4) as Ep,
            tc.tile_pool(name="yp", bufs=1) as yp,
            tc.tile_pool(name="ps", bufs=3, space="PSUM") as ps,
            tc.tile_pool(name="pso", bufs=1, space="PSUM") as pso,
        ):
            wk_sb = const.tile([E, E], F16, tag="wk")
            wq_sb = const.tile([E, E], F16, tag="wq")
            wv_sb = const.tile([E, E], F16, tag="wv")
            wu_sb = const.tile([E, E], F16, tag="wu")
            id_sb = const.tile([128, 128], F16, tag="id")
            sel_sb = const.tile([128, E], F16, tag="sel")
            bq_sb = const.tile([E, 1], F32, tag="bq")
            bf_sb = const.tile([E, 1], F32, tag="bf")
            nc.sync.dma_start(wk_sb[:], d_wkT.ap()[:])
            nc.sync.dma_start(wq_sb[:], d_wqT.ap()[:])
            nc.sync.dma_start(wv_sb[:], d_wvT.ap()[:])
            nc.sync.dma_start(wu_sb[:], d_wuT.ap()[:])
            nc.sync.dma_start(id_sb[:], d_id.ap()[:])
            nc.sync.dma_start(sel_sb[:], d_sel.ap()[:])
            nc.sync.dma_start(bq_sb[:], d_bq.ap()[:])
            nc.sync.dma_start(bf_sb[:], d_bf.ap()[:])

            K_sb, Q_sb, VT_sb, out_sb, R_sb, R32_sb, RD_sb = {}, {}, {}, {}, {}, {}, {}
            for b in range(BPC):
                K_sb[b] = persist.tile([E, N], F16, tag=f"K{b}", name=f"K{b}")
                Q_sb[b] = persist.tile([E, N], F16, tag=f"Q{b}", name=f"Q{b}")
                VT_sb[b] = persist.tile([128, NT, H, HC + 1], F16, tag=f"VT{b}", name=f"VT{b}")
                out_sb[b] = persist.tile([E, N], F16, tag=f"O{b}", name=f"O{b}")
                R_sb[b] = persist.tile([128, N], F16, tag=f"R{b}", name=f"R{b}")
                R32_sb[b] = persist.tile([128, N], F32, tag=f"R32{b}", name=f"R32{b}")
                RD_sb[b] = persist.tile([128, N], F32, tag=f"RD{b}", name=f"RD{b}")

            # ---- projections ----
            for b in range(BPC):
                x_sb = xp.tile([E, N], F16, tag="x")
                nc.sync.dma_start(x_sb[:], d_x2.ap()[b])
                nc.vector.memset(VT_sb[b][:], 1.0)
                nc.vector.memset(R_sb[b][:], 0.0)
                nc.vector.memset(RD_sb[b][:], 1.0)
                for j in range(2):
                    js = ds(512 * j, 512)
                    pk = ps.tile([128, 512], F32, tag="ps")
                    nc.tensor.matmul(pk[:], wk_sb[:], x_sb[:, js],
                                     start=True, stop=True)
                    nc.vector.tensor_copy(K_sb[b][:, js], pk[:])
                    pq = ps.tile([128, 512], F32, tag="ps")
                    nc.tensor.matmul(pq[:], wq_sb[:], x_sb[:, js],
                                     start=True, stop=True)
                    nc.vector.tensor_scalar_add(Q_sb[b][:, js], pq[:], bq_sb[:])
                for t in range(NT):
                    pv = ps.tile([128, 128], F32, tag="ps")
                    nc.tensor.matmul(pv[:], x_sb[:, ts(t, 128)], wv_sb[:],
                                     start=True, stop=True)
                    nc.vector.tensor_copy(
                        VT_sb[b][:, t, :, 0:HC],
                        pv[:].rearrange("p (h c) -> p h c", h=H),
                    )

            # ---- attention, head pairs ----
            for p in range(2 if stage >= 2 else 0):
                hs = (2 * p, 2 * p + 1)
                rel_t = {}
                for h in hs:
                    rel_t[h] = relp.tile([128, NT, N], F16, tag="rel", name=f"rel{h}")
                    for t in range(NT):
                        nc.sync.dma_start(rel_t[h][:, t, :], d_rel.ap()[h, t])
                for b in range(BPC):
                    Et = {h: Ep.tile([128, NT, N], F16, tag="E", name=f"E{h}") for h in hs}
                    for t in range(NT):
                        pa = {}
                        for h in hs:
                            pa[h] = ps.tile([128, N], F32, tag="ps",
                                            name=f"pa{h}")
                        for j in range(2):
                            js = ds(512 * j, 512)
                            for h in hs:
                                nc.tensor.matmul(
                                    pa[h][:, js], id_sb[:], rel_t[h][:, t, js],
                                    start=True, stop=False,
                                )
                            for h in hs:
                                nc.tensor.matmul(
                                    pa[h][:, js],
                                    K_sb[b][ds(HC * h, HC), ts(t, 128)],
                                    Q_sb[b][ds(HC * h, HC), js],
                                    start=False, stop=True,
                                    tile_position=(HC * h, 0),
                                )
                        for h in hs:
                            nc.scalar.activation(Et[h][:, t, :], pa[h][:], AF.Exp)
                    # second matmul: out_h^num / D, heads separately
                    for h in (hs if stage >= 3 else ()):
                        po = pso.tile([HC + 1, N], F32, tag="pso")
                        for j in range(2):
                            js = ds(512 * j, 512)
                            for t in range(NT):
                                nc.tensor.matmul(
                                    po[:, js],
                                    VT_sb[b][:, t, h, :],
                                    Et[h][:, t, js],
                                    start=(t == 0), stop=(t == NT - 1),
                                )
                        nc.vector.tensor_copy(RD_sb[b][ds(32 * h, 1), :], po[HC:HC + 1, :])
                        nc.vector.tensor_copy(out_sb[b][ds(HC * h, HC), :],
                                              po[0:HC, :])

            # ---- divide + output projection ----
            sub = int(os.environ.get("KSUB", "4"))
            for b in range(BPC if stage >= 4 else 0):
                nc.vector.reciprocal_approx_fast(out=R32_sb[b][:], in_=RD_sb[b][:])
                nc.vector.tensor_copy(R_sb[b][:], R32_sb[b][:])
                pbc = pso.tile([128, N], F32, tag="pso")
                if sub >= 1:
                    for j in range(2):
                        js = ds(512 * j, 512)
                        nc.tensor.matmul(pbc[:, js], sel_sb[:], R_sb[b][:, js],
                                         start=True, stop=True)
                if sub >= 2:
                    nc.vector.tensor_mul(out_sb[b][:], out_sb[b][:], pbc[:])
                py = pso.tile([128, N], F32, tag="pso")
                if sub >= 3:
                    for j in range(2):
                        js = ds(512 * j, 512)
                        nc.tensor.matmul(py[:, js], wu_sb[:], out_sb[b][:, js],
                                         start=True, stop=True)
                    y_sb = yp.tile([E, N], F32, tag="y")
                    nc.vector.tensor_scalar_add(y_sb[:], py[:], bf_sb[:])
                    nc.sync.dma_start(d_y2.ap()[b], y_sb[:])

            if stage < 4 or int(os.environ.get("KSUB", "4")) < 3:
                for b in range(BPC):
                    nc.gpsimd.dma_start(d_y2.ap()[b], K_sb[b][:])

    nc.compile()
    _CACHE[key] = nc
    return nc


def kernel(x, Wk, bk, Wq, bq, Wv, bv, Wu, bu, pos_enc):
    global LAST_RESULT
    x = np.ascontiguousarray(np.asarray(x, np.float32))
    Wk = np.asarray(Wk, np.float32)
    Wq = np.asarray(Wq, np.float32)
    Wv = np.asarray(Wv, np.float32)
    Wu = np.asarray(Wu, np.float32)
    bq = np.asarray(bq, np.float32)
    bv = np.asarray(bv, np.float32)
    bu = np.asarray(bu, np.float32)
    pos_enc = np.asarray(pos_enc, np.float32)

    wkT = np.ascontiguousarray(Wk.T.astype(np.float16))
    wqT = np.ascontiguousarray((Wq * SCALE).T.astype(np.float16))
    wvT = np.ascontiguousarray(Wv.T.astype(np.float16))
    wuT = np.ascontiguousarray(Wu.T.astype(np.float16))
    bqv = np.ascontiguousarray((bq * SCALE).reshape(E, 1))
    bfv = np.ascontiguousarray((Wu @ bv + bu).reshape(E, 1))

    idx = _rel_indices(NY, NX)
    rel = pos_enc[:, idx]                         # (H, N, N) fp32
    relb = np.ascontiguousarray(
        rel.reshape(H, NT, 128, N).astype(np.float16))
    ident = np.eye(128, dtype=np.float16)
    sel4 = np.zeros((128, E), np.float16)
    for h in range(H):
        sel4[32 * h, HC * h:HC * (h + 1)] = 1.0

    nc = _build()

    common = dict(wkT=wkT, wqT=wqT, wvT=wvT, wuT=wuT, bqv=bqv, bfv=bfv,
                  relb=relb, ident=ident, sel4=sel4)
    in_maps = []
    xr = x.reshape(B, E, N)
    for c in range(NCORES):
        m = dict(common)
        m["x2"] = np.ascontiguousarray(xr[BPC * c:BPC * (c + 1)].astype(np.float16))
        in_maps.append(m)

    trace = os.environ.get("BASS_TRACE", "") not in ("", "0")
    if trace:
        _ensure_ntff_hook()
    res = bass_utils.run_bass_kernel_spmd(
        nc, in_maps, core_ids=list(range(NCORES)), trace=trace)
    LAST_RESULT = res

    y = np.empty((B, E, N), np.float32)
    for c in range(NCORES):
        y[BPC * c:BPC * (c + 1)] = res.results[c]["y2"]
    return y.reshape(B, E, NY, NX)
